# revision 57
# baseline (speedup 1.0000x reference)
"""Trainium2 Bass kernel for nn_LinearNNEncoder (fused Linear+GELU, masked
per-batch mean/std over ragged sequences), data-parallel over 8 NeuronCores.

Contract: kernel(**inputs) takes the FULL inputs (x [64,2048,300] f32,
W [300,300], b [300]) and returns the FULL output [64, 600] f32
(concat(std, mean) per batch).

Strategy per core (8 batches of 2048 tokens each, 128-token tiles):
  - GEMM in fp8 (e4m3) DoubleRow mode at 0.5 PE-cycles/row: x is
    host-transposed/cast into k-major fp8 tiles, W is host-scaled by 16 into
    fp8 range (the GELU applies scale=1/16 on its PSUM input).  Contraction
    k=0..301 runs as two DoubleRow matmuls (k 0..255 on 128 partitions, the
    rest on 23 partitions); the bias rides the two spare rows of the second
    matmul as ones-rows against fp8 error-feedback weights
    (fp8(16b) + fp8(16b - fp8(16b)), ~0.1% error).
  - One exact-GELU and one square per TILE PAIR: the two GEMM outputs live in
    one [128, 2, 512] PSUM tile (two bank-aligned halves), so a strided
    [128,2,300] AP lets a single ACT/DVE op cover both tiles, halving the
    fixed per-op access cost.  y/y^2 stream in bf16.
  - Stats as ones-stationary matmuls accumulating in PSUM: per-batch
    Q = sum(y^2) per tile (2 tiles behind the GEMM) and S = sum(y) per pair
    (on a DVE pair-add of the y halves, one pair behind), so no PE wait ever
    exceeds the ACT->DVE chain.  Raw S|Q drain straight to DRAM (DVE copies +
    Pool-queue DMAs so the SP queue only carries the x prefetch stream).
  - No per-token padding mask: a padded row is the constant (-1,...,-1), so
    its y is a constant c.  One all-pad tile runs through the exact same
    GEMM->GELU->square pipeline once ("cpad"), making the correction constant
    bitwise-identical to the streamed padded rows (any fp8/bf16 quantization
    cancels exactly).  c|c^2 is DMA'd out, n_pad is counted on the host
    (x[:,:,0] == -1.0; exact-f32 compare -- NEVER compare f32r on DVE, it
    quantizes to ~1e-4), and the host finishes the epilogue in f64:
    sum_valid = S - n_pad*c, mean/unbiased-var/sqrt on [64, 600].
Engine budget per core (TimelineSim): ACT 46us (pair-GELUs) ~= DVE 45us
(pair-squares + adds + drains) > PE 40us (matmuls) > HWDGE/DMA.
"""
import numpy as np

B, T, D = 64, 2048, 300
NCORES = 8
B_LOC = B // NCORES     # batches per core
TPB = T // 128          # token tiles per batch (16)
G = 8                   # token tiles per DMA group
GPB = TPB // G          # groups per batch (4)
NG = B_LOC * GPB        # groups per core (32)
KB = 23                 # second DoubleRow matmul: k = 256 + s*23 + kp
WS = 16.0               # W is scaled by WS into fp8 range; GELU applies 1/WS
SCI = 1.0 / WS

_cache = {}


def _build_nc():
    from contextlib import ExitStack
    import concourse.tile as tile
    from concourse import mybir, bacc

    f32 = mybir.dt.float32
    f32r = mybir.dt.float32r
    bf16 = mybir.dt.bfloat16
    AF = mybir.ActivationFunctionType
    OP = mybir.AluOpType

    fp8 = mybir.dt.float8e4
    PM = mybir.MatmulPerfMode

    nc = bacc.Bacc("TRN2", target_bir_lowering=False, debug=False)
    xta_dram = nc.dram_tensor("xta", [NG, 128, G * 2 * 128], fp8, kind="ExternalInput")
    xtb_dram = nc.dram_tensor("xtb", [NG, KB, G * 2 * 128], fp8, kind="ExternalInput")
    w8_dram = nc.dram_tensor("w8", [128, 2 * 2 * D], fp8, kind="ExternalInput")
    xpa_dram = nc.dram_tensor("xpa", [128, 2 * 128], fp8, kind="ExternalInput")
    xpb_dram = nc.dram_tensor("xpb", [KB, 2 * 128], fp8, kind="ExternalInput")
    on_dram = nc.dram_tensor("on", [128, 1], bf16, kind="ExternalInput")
    out_dram = nc.dram_tensor("out", [B_LOC, 2 * D], f32, kind="ExternalOutput")
    ccv_dram = nc.dram_tensor("ccv", [1, 2 * D], f32, kind="ExternalOutput")

    xta_ap = xta_dram.ap().rearrange("s p (g c t) -> s p g c t", g=G, c=2)
    xtb_ap = xtb_dram.ap().rearrange("s p (g c t) -> s p g c t", g=G, c=2)

    with ExitStack() as ctx:
        tc = ctx.enter_context(tile.TileContext(nc))
        const = ctx.enter_context(tc.tile_pool(name="const", bufs=1))
        xgp = ctx.enter_context(tc.tile_pool(name="xgp", bufs=4))
        xgbp = ctx.enter_context(tc.tile_pool(name="xgbp", bufs=4))
        yyp = ctx.enter_context(tc.tile_pool(name="yyp", bufs=6))
        prp = ctx.enter_context(tc.tile_pool(name="prp", bufs=6))

        drp = ctx.enter_context(tc.tile_pool(name="drp", bufs=2))
        epil = ctx.enter_context(tc.tile_pool(name="epil", bufs=1))
        ps_y = ctx.enter_context(tc.tile_pool(name="ps_y", bufs=2, space="PSUM"))
        ps_s = ctx.enter_context(tc.tile_pool(name="ps_s", bufs=2, space="PSUM"))
        ps_q = ctx.enter_context(tc.tile_pool(name="ps_q", bufs=2, space="PSUM"))

        xg_tiles = {}

        def issue_xg(s):
            xga = xgp.tile([128, G, 2, 128], fp8, name=f"xga_{s}", tag="xga")
            nc.sync.dma_start(xga[:], xta_ap[s])
            xgb = xgbp.tile([KB, G, 2, 128], fp8, name=f"xgb_{s}", tag="xgb")
            nc.sync.dma_start(xgb[:], xtb_ap[s])
            xg_tiles[s] = (xga, xgb)

        def issue_dma(s):
            issue_xg(s)

        # first group in half-DMAs: the first GEMM starts sooner
        xg0a = xgp.tile([128, G, 2, 128], fp8, name="xga_0", tag="xga")
        xg0b = xgbp.tile([KB, G, 2, 128], fp8, name="xgb_0", tag="xgb")
        nc.sync.dma_start(xg0a[:, 0:G // 2], xta_ap[0][:, 0:G // 2])
        w8_sb = const.tile([128, 2, 2, D], fp8)
        nc.sync.dma_start(
            w8_sb[:], w8_dram.ap().rearrange("p (m c o) -> p m c o", m=2, c=2))
        nc.sync.dma_start(xg0b[:], xtb_ap[0])
        nc.sync.dma_start(xg0a[:, G // 2:G], xta_ap[0][:, G // 2:G])
        xg_tiles[0] = (xg0a, xg0b)
        issue_xg(1)
        ones = const.tile([128, 1], bf16)
        nc.sync.dma_start(ones[:], on_dram.ap())
        xpa_sb = const.tile([128, 2, 128], fp8)
        nc.sync.dma_start(xpa_sb[:], xpa_dram.ap().rearrange("p (c t) -> p c t", c=2))
        xpb_sb = const.tile([KB, 2, 128], fp8)
        nc.sync.dma_start(xpb_sb[:], xpb_dram.ap().rearrange("p (c t) -> p c t", c=2))
        cyy = const.tile([128, 2 * D], bf16)
        cc32 = const.tile([1, 2 * D], f32)

        cur = {}
        yy_tiles = {}
        pr_tiles = {}
        PPB = TPB // 2       # tile pairs per batch (8)
        NP = NG * G // 2     # tile pairs per core (64)
        NT = NG * G          # token tiles per core

        def qstat(t):
            """Q-sum matmul for tile t (2 tiles behind the GEMM).  Per-tile
            on the pair tile's square halves; the chain is pair-gelu ->
            pair-square."""
            p, par = divmod(t, 2)
            yy = yy_tiles[p] if par == 0 else yy_tiles.pop(p)
            bs, jt = divmod(t, TPB)
            if jt == 0:
                cur["q"] = ps_q.tile([1, D], f32, name=f"ps_q_{bs}", tag="q")
            nc.tensor.matmul(cur["q"][0:1, 0:D], ones[:], yy[:, par, D:2 * D],
                             start=(jt == 0), stop=(jt == TPB - 1))

        def sstat(p):
            """S-sum matmul for tile pair p (a pair behind the y-half add,
            whose chain avoids the squares entirely)."""
            yp2 = pr_tiles.pop(p)
            bs, jp = divmod(p, PPB)
            if jp == 0:
                cur["s"] = ps_s.tile([1, D], f32, name=f"ps_s_{bs}", tag="s")
            nc.tensor.matmul(cur["s"][0:1, 0:D], ones[:], yp2[:],
                             start=(jp == 0), stop=(jp == PPB - 1))

        def drain(bs):
            dr = drp.tile([1, 2 * D], f32, name=f"dr_{bs}", tag="dr")
            nc.vector.tensor_copy(dr[0:1, 0:D], cur["s"][0:1, 0:D])
            nc.vector.tensor_copy(dr[0:1, D:2 * D], cur["q"][0:1, 0:D])
            # straight to DRAM: the host does the epilogue (mean/std) in
            # f64.  Pool-queue DMAs keep the SP queue free for the xg
            # prefetch stream; the last batch takes the faster HWDGE path.
            if bs == B_LOC - 1:
                nc.sync.dma_start(out_dram.ap()[bs:bs + 1, :], dr[0:1, :])
            else:
                nc.gpsimd.dma_start(out_dram.ap()[bs:bs + 1, :], dr[0:1, :])

        for s in range(NG):
            if s + 2 < NG:
                issue_dma(s + 2)
            xga, xgb = xg_tiles.pop(s)

            for t in range(G):
                gidx = s * G + t
                par = gidx % 2
                if par == 0:
                    pypr = ps_y.tile([128, 2, 512], f32,
                                     name=f"py_{gidx // 2}", tag="py")
                    cur["py"] = pypr
                else:
                    pypr = cur["py"]
                nc.tensor.matmul(pypr[:, par, 0:D], xga[:, t, :, :],
                                 w8_sb[:, 0, :, :],
                                 start=True, stop=False, perf_mode=PM.DoubleRow)
                nc.tensor.matmul(pypr[:, par, 0:D], xgb[:, t, :, :],
                                 w8_sb[0:KB, 1, :, :],
                                 start=False, stop=True, perf_mode=PM.DoubleRow)
                if par == 1:
                    p = gidx // 2
                    # one GELU + one square for the pair: the [128,2,300] AP
                    # spans the pair tile's two (bank-aligned) PSUM banks,
                    # halving ACT's fixed per-op access cost
                    yy = yyp.tile([128, 2, 2 * D], bf16, name=f"yy_{p}",
                                  tag="yy")
                    nc.scalar.activation(yy[:, :, 0:D], pypr[:, :, 0:D],
                                         AF.Gelu, scale=SCI)
                    nc.vector.tensor_mul(yy[:, :, D:2 * D], yy[:, :, 0:D],
                                         yy[:, :, 0:D])
                    yy_tiles[p] = yy
                    yp2 = prp.tile([128, D], bf16, name=f"yp2_{p}", tag="yp2")
                    nc.vector.tensor_add(yp2[:], yy[:, 0, 0:D], yy[:, 1, 0:D])
                    pr_tiles[p] = yp2
                if gidx >= 2:
                    qstat(gidx - 2)
                if par == 1 and gidx // 2 >= 1:
                    sstat(gidx // 2 - 1)
                    bs, jt = divmod(gidx - 2, TPB)
                    if jt == TPB - 1:
                        drain(bs)
            if s == 1:
                # device-side padded-row constant: one all-pad tile through
                # the exact same GEMM -> GELU -> square pipeline so c matches
                # padded-row outputs bitwise (emitted after group 0 so the
                # main GEMM stream starts as soon as xg0/w3 land; also
                # preloads the Sqrt ACT table during the main loop)
                pyc = ps_y.tile([128, 2, 512], f32, name="pyc", tag="py")
                nc.tensor.matmul(pyc[:, 0, 0:D], xpa_sb[:], w8_sb[:, 0, :, :],
                                 start=True, stop=False, perf_mode=PM.DoubleRow)
                nc.tensor.matmul(pyc[:, 0, 0:D], xpb_sb[:], w8_sb[0:KB, 1, :, :],
                                 start=False, stop=True, perf_mode=PM.DoubleRow)
                nc.scalar.activation(cyy[:, 0:D], pyc[:, 0, 0:D], AF.Gelu,
                                     scale=SCI)
                nc.vector.tensor_mul(cyy[:, D:2 * D], cyy[:, 0:D], cyy[:, 0:D])
                nc.scalar.copy(cc32[0:1, :], cyy[0:1, :])
                nc.gpsimd.dma_start(ccv_dram.ap()[:], cc32[0:1, :])
        qstat(NT - 2)
        qstat(NT - 1)
        sstat(NP - 1)
        drain(B_LOC - 1)

    nc.compile()
    return nc


def _prep_inputs(x, W, b):
    """Host prep: k-transpose x into grouped tiles, pack W^T k-tiles + bias row,
    precompute the padded-row GELU constant c."""
    import ml_dtypes
    fp8 = ml_dtypes.float8_e4m3fn
    bft = ml_dtypes.bfloat16
    x = np.ascontiguousarray(x, np.float32)
    W = np.asarray(W, np.float32)
    b = np.asarray(b, np.float32)

    x8 = x.astype(fp8)
    xr8 = x8.reshape(B, GPB, G, 128, D)  # [b,grp,g,tok,k]
    # m0: k = s*128 + kp  (k 0..255)
    xta = np.ascontiguousarray(
        xr8[..., 0:256].reshape(B, GPB, G, 128, 2, 128)
        .transpose(0, 1, 5, 2, 4, 3))    # [b,grp,kp,g,s,tok]
    # m1: k = 256 + s*KB + kp (kp<KB); k==300/301 -> 0 (bias handled in bf16)
    xtb = np.zeros((B, GPB, KB, G, 2, 128), fp8)
    xtb[:, :, :, :, 0, :] = xr8[..., 256:256 + KB].transpose(0, 1, 4, 2, 3)
    xtb[:, :, 0:D - 256 - KB, :, 1, :] = (
        xr8[..., 256 + KB:D].transpose(0, 1, 4, 2, 3))
    xtb[:, :, KB - 2:KB, :, 1, :] = fp8(1.0)   # bias ones rows
    shards_a = [
        np.ascontiguousarray(
            xta[c * B_LOC:(c + 1) * B_LOC].reshape(NG, 128, G * 2 * 128))
        for c in range(NCORES)
    ]
    shards_b = [
        np.ascontiguousarray(
            xtb[c * B_LOC:(c + 1) * B_LOC].reshape(NG, KB, G * 2 * 128))
        for c in range(NCORES)
    ]
    npad = (x[:, :, 0] == -1.0).sum(axis=1).astype(np.float64)  # [B]

    w16 = (W.T * WS).astype(fp8)      # [k, o], scaled into fp8 range
    w8 = np.zeros((128, 2, 2, D), fp8)
    w8[:, 0, 0, :] = w16[0:128]
    w8[:, 0, 1, :] = w16[128:256]
    w8[0:KB, 1, 0, :] = w16[256:256 + KB]
    w8[0:D - 256 - KB, 1, 1, :] = w16[256 + KB:D]
    # bias rides the two spare m1 rows (k "300"/"301") with fp8
    # error-feedback: b ~ fp8(WS*b) + fp8(WS*b - fp8(WS*b)), ~0.13% error
    b0 = (b * WS).astype(fp8)
    w8[KB - 2, 1, 1, :] = b0
    w8[KB - 1, 1, 1, :] = (b * WS - b0.astype(np.float64)).astype(fp8)

    # the all-padded-row tile: k<300 -> -1; ones on the two bias rows
    xpa = np.full((128, 2, 128), fp8(-1.0), fp8).reshape(128, 256)
    xpb = np.zeros((KB, 2, 128), fp8)
    xpb[:, 0, :] = fp8(-1.0)
    xpb[0:D - 256 - KB, 1, :] = fp8(-1.0)
    xpb[KB - 2:KB, 1, :] = fp8(1.0)
    xpb = xpb.reshape(KB, 256)
    return (shards_a, shards_b, w8.reshape(128, 2 * 2 * D), xpa, xpb, npad)


def kernel(x, W, b):
    from concourse.bass_utils import run_bass_kernel_spmd

    if "nc" not in _cache:
        _cache["nc"] = _build_nc()
    nc = _cache["nc"]

    import ml_dtypes
    sa, sb, w8, xpa, xpb, npad = _prep_inputs(x, W, b)
    on = np.ones((128, 1), ml_dtypes.bfloat16)
    in_maps = [{"xta": sa[c], "xtb": sb[c], "w8": w8,
                "xpa": xpa, "xpb": xpb, "on": on}
               for c in range(NCORES)]
    res = run_bass_kernel_spmd(nc, in_maps, core_ids=list(range(NCORES)))
    sums = np.concatenate(
        [res.results[c]["out"] for c in range(NCORES)], axis=0
    ).astype(np.float64)                      # [B, 600] = S | Q (unmasked)
    cc = np.concatenate(
        [np.repeat(res.results[c]["ccv"].astype(np.float64), B_LOC, axis=0)
         for c in range(NCORES)], axis=0)     # [B, 600] = c | c^2 per core
    n = (T - npad)[:, None]
    sv = sums - npad[:, None] * cc            # valid-token S | Q
    mean = sv[:, 0:D] / n
    var = (sv[:, D:2 * D] - n * mean * mean) / np.maximum(n - 1.0, 1.0)
    std = np.sqrt(np.maximum(var, 0.0))
    return np.concatenate([std, mean], axis=1).astype(np.float32)


# revision 58
# speedup vs baseline: 1.0013x; 1.0013x over previous
"""Trainium2 Bass kernel for nn_LinearNNEncoder (fused Linear+GELU, masked
per-batch mean/std over ragged sequences), data-parallel over 8 NeuronCores.

Contract: kernel(**inputs) takes the FULL inputs (x [64,2048,300] f32,
W [300,300], b [300]) and returns the FULL output [64, 600] f32
(concat(std, mean) per batch).

Strategy per core (8 batches of 2048 tokens each, 128-token tiles):
  - GEMM in fp8 (e4m3) DoubleRow mode at 0.5 PE-cycles/row: x is
    host-transposed/cast into k-major fp8 tiles, W is host-scaled by 16 into
    fp8 range (the GELU applies scale=1/16 on its PSUM input).  Contraction
    k=0..301 runs as two DoubleRow matmuls (k 0..255 on 128 partitions, the
    rest on 23 partitions); the bias rides the two spare rows of the second
    matmul as ones-rows against fp8 error-feedback weights
    (fp8(16b) + fp8(16b - fp8(16b)), ~0.1% error).
  - One exact-GELU and one square per TILE PAIR: the two GEMM outputs live in
    one [128, 2, 512] PSUM tile (two bank-aligned halves), so a strided
    [128,2,300] AP lets a single ACT/DVE op cover both tiles, halving the
    fixed per-op access cost.  y/y^2 stream in bf16.
  - Stats as ones-stationary matmuls accumulating in PSUM: per-batch
    Q = sum(y^2) per tile (2 tiles behind the GEMM) and S = sum(y) per pair
    (on a DVE pair-add of the y halves, one pair behind), so no PE wait ever
    exceeds the ACT->DVE chain.  Raw S|Q drain straight to DRAM (DVE copies +
    Pool-queue DMAs so the SP queue only carries the x prefetch stream).
  - No per-token padding mask: a padded row is the constant (-1,...,-1), so
    its y is a constant c.  One all-pad tile runs through the exact same
    GEMM->GELU->square pipeline once ("cpad"), making the correction constant
    bitwise-identical to the streamed padded rows (any fp8/bf16 quantization
    cancels exactly).  c|c^2 is DMA'd out, n_pad is counted on the host
    (x[:,:,0] == -1.0; exact-f32 compare -- NEVER compare f32r on DVE, it
    quantizes to ~1e-4), and the host finishes the epilogue in f64:
    sum_valid = S - n_pad*c, mean/unbiased-var/sqrt on [64, 600].
Engine budget per core (TimelineSim): ACT 46us (pair-GELUs) ~= DVE 45us
(pair-squares + adds + drains) > PE 40us (matmuls) > HWDGE/DMA.
"""
import numpy as np

B, T, D = 64, 2048, 300
NCORES = 8
B_LOC = B // NCORES     # batches per core
TPB = T // 128          # token tiles per batch (16)
G = 8                   # token tiles per DMA group
GPB = TPB // G          # groups per batch (4)
NG = B_LOC * GPB        # groups per core (32)
KB = 23                 # second DoubleRow matmul: k = 256 + s*23 + kp
WS = 16.0               # W is scaled by WS into fp8 range; GELU applies 1/WS
SCI = 1.0 / WS

_cache = {}


def _build_nc():
    from contextlib import ExitStack
    import concourse.tile as tile
    from concourse import mybir, bacc

    f32 = mybir.dt.float32
    f32r = mybir.dt.float32r
    bf16 = mybir.dt.bfloat16
    AF = mybir.ActivationFunctionType
    OP = mybir.AluOpType

    fp8 = mybir.dt.float8e4
    PM = mybir.MatmulPerfMode

    nc = bacc.Bacc("TRN2", target_bir_lowering=False, debug=False)
    xta_dram = nc.dram_tensor("xta", [NG, 128, G * 2 * 128], fp8, kind="ExternalInput")
    xtb_dram = nc.dram_tensor("xtb", [NG, KB, G * 2 * 128], fp8, kind="ExternalInput")
    w8_dram = nc.dram_tensor("w8", [128, 2 * 2 * D], fp8, kind="ExternalInput")
    xpa_dram = nc.dram_tensor("xpa", [128, 2 * 128], fp8, kind="ExternalInput")
    xpb_dram = nc.dram_tensor("xpb", [KB, 2 * 128], fp8, kind="ExternalInput")
    on_dram = nc.dram_tensor("on", [128, 1], bf16, kind="ExternalInput")
    out_dram = nc.dram_tensor("out", [B_LOC, 2 * D], f32, kind="ExternalOutput")
    ccv_dram = nc.dram_tensor("ccv", [1, 2 * D], f32, kind="ExternalOutput")

    xta_ap = xta_dram.ap().rearrange("s p (g c t) -> s p g c t", g=G, c=2)
    xtb_ap = xtb_dram.ap().rearrange("s p (g c t) -> s p g c t", g=G, c=2)

    with ExitStack() as ctx:
        tc = ctx.enter_context(tile.TileContext(nc))
        const = ctx.enter_context(tc.tile_pool(name="const", bufs=1))
        xgp = ctx.enter_context(tc.tile_pool(name="xgp", bufs=4))
        xgbp = ctx.enter_context(tc.tile_pool(name="xgbp", bufs=4))
        yyp = ctx.enter_context(tc.tile_pool(name="yyp", bufs=8))
        prp = ctx.enter_context(tc.tile_pool(name="prp", bufs=6))

        drp = ctx.enter_context(tc.tile_pool(name="drp", bufs=2))
        epil = ctx.enter_context(tc.tile_pool(name="epil", bufs=1))
        ps_y = ctx.enter_context(tc.tile_pool(name="ps_y", bufs=2, space="PSUM"))
        ps_s = ctx.enter_context(tc.tile_pool(name="ps_s", bufs=2, space="PSUM"))
        ps_q = ctx.enter_context(tc.tile_pool(name="ps_q", bufs=2, space="PSUM"))

        xg_tiles = {}

        def issue_xg(s):
            xga = xgp.tile([128, G, 2, 128], fp8, name=f"xga_{s}", tag="xga")
            nc.sync.dma_start(xga[:], xta_ap[s])
            xgb = xgbp.tile([KB, G, 2, 128], fp8, name=f"xgb_{s}", tag="xgb")
            nc.sync.dma_start(xgb[:], xtb_ap[s])
            xg_tiles[s] = (xga, xgb)

        def issue_dma(s):
            issue_xg(s)

        # first group in half-DMAs: the first GEMM starts sooner
        xg0a = xgp.tile([128, G, 2, 128], fp8, name="xga_0", tag="xga")
        xg0b = xgbp.tile([KB, G, 2, 128], fp8, name="xgb_0", tag="xgb")
        nc.sync.dma_start(xg0a[:, 0:G // 2], xta_ap[0][:, 0:G // 2])
        w8_sb = const.tile([128, 2, 2, D], fp8)
        nc.sync.dma_start(
            w8_sb[:], w8_dram.ap().rearrange("p (m c o) -> p m c o", m=2, c=2))
        nc.sync.dma_start(xg0b[:], xtb_ap[0])
        nc.sync.dma_start(xg0a[:, G // 2:G], xta_ap[0][:, G // 2:G])
        xg_tiles[0] = (xg0a, xg0b)
        issue_xg(1)
        ones = const.tile([128, 1], bf16)
        nc.sync.dma_start(ones[:], on_dram.ap())
        xpa_sb = const.tile([128, 2, 128], fp8)
        nc.sync.dma_start(xpa_sb[:], xpa_dram.ap().rearrange("p (c t) -> p c t", c=2))
        xpb_sb = const.tile([KB, 2, 128], fp8)
        nc.sync.dma_start(xpb_sb[:], xpb_dram.ap().rearrange("p (c t) -> p c t", c=2))
        cyy = const.tile([128, 2 * D], bf16)
        cc32 = const.tile([1, 2 * D], f32)

        cur = {}
        yy_tiles = {}
        pr_tiles = {}
        PPB = TPB // 2       # tile pairs per batch (8)
        NP = NG * G // 2     # tile pairs per core (64)
        NT = NG * G          # token tiles per core

        def qstat(t):
            """Q-sum matmul for tile t (2 tiles behind the GEMM).  Per-tile
            on the pair tile's square halves; the chain is pair-gelu ->
            pair-square."""
            p, par = divmod(t, 2)
            yy = yy_tiles[p] if par == 0 else yy_tiles.pop(p)
            bs, jt = divmod(t, TPB)
            if jt == 0:
                cur["q"] = ps_q.tile([1, D], f32, name=f"ps_q_{bs}", tag="q")
            nc.tensor.matmul(cur["q"][0:1, 0:D], ones[:], yy[:, par, D:2 * D],
                             start=(jt == 0), stop=(jt == TPB - 1))

        def sstat(p):
            """S-sum matmul for tile pair p (a pair behind the y-half add,
            whose chain avoids the squares entirely)."""
            yp2 = pr_tiles.pop(p)
            bs, jp = divmod(p, PPB)
            if jp == 0:
                cur["s"] = ps_s.tile([1, D], f32, name=f"ps_s_{bs}", tag="s")
            nc.tensor.matmul(cur["s"][0:1, 0:D], ones[:], yp2[:],
                             start=(jp == 0), stop=(jp == PPB - 1))

        def drain(bs):
            dr = drp.tile([1, 2 * D], f32, name=f"dr_{bs}", tag="dr")
            nc.vector.tensor_copy(dr[0:1, 0:D], cur["s"][0:1, 0:D])
            nc.vector.tensor_copy(dr[0:1, D:2 * D], cur["q"][0:1, 0:D])
            # straight to DRAM: the host does the epilogue (mean/std) in
            # f64.  Pool-queue DMAs keep the SP queue free for the xg
            # prefetch stream; the last batch takes the faster HWDGE path.
            if bs == B_LOC - 1:
                nc.sync.dma_start(out_dram.ap()[bs:bs + 1, :], dr[0:1, :])
            else:
                nc.gpsimd.dma_start(out_dram.ap()[bs:bs + 1, :], dr[0:1, :])

        for s in range(NG):
            if s + 2 < NG:
                issue_dma(s + 2)
            xga, xgb = xg_tiles.pop(s)

            for t in range(G):
                gidx = s * G + t
                par = gidx % 2
                if par == 0:
                    pypr = ps_y.tile([128, 2, 512], f32,
                                     name=f"py_{gidx // 2}", tag="py")
                    cur["py"] = pypr
                else:
                    pypr = cur["py"]
                nc.tensor.matmul(pypr[:, par, 0:D], xga[:, t, :, :],
                                 w8_sb[:, 0, :, :],
                                 start=True, stop=False, perf_mode=PM.DoubleRow)
                nc.tensor.matmul(pypr[:, par, 0:D], xgb[:, t, :, :],
                                 w8_sb[0:KB, 1, :, :],
                                 start=False, stop=True, perf_mode=PM.DoubleRow)
                if par == 1:
                    p = gidx // 2
                    # one GELU + one square for the pair: the [128,2,300] AP
                    # spans the pair tile's two (bank-aligned) PSUM banks,
                    # halving ACT's fixed per-op access cost
                    yy = yyp.tile([128, 2, 2 * D], bf16, name=f"yy_{p}",
                                  tag="yy")
                    nc.scalar.activation(yy[:, :, 0:D], pypr[:, :, 0:D],
                                         AF.Gelu, scale=SCI)
                    nc.vector.tensor_mul(yy[:, :, D:2 * D], yy[:, :, 0:D],
                                         yy[:, :, 0:D])
                    yy_tiles[p] = yy
                    yp2 = prp.tile([128, D], bf16, name=f"yp2_{p}", tag="yp2")
                    nc.vector.tensor_add(yp2[:], yy[:, 0, 0:D], yy[:, 1, 0:D])
                    pr_tiles[p] = yp2
                if gidx >= 2:
                    qstat(gidx - 2)
                if par == 1 and gidx // 2 >= 1:
                    sstat(gidx // 2 - 1)
                    bs, jt = divmod(gidx - 2, TPB)
                    if jt == TPB - 1:
                        drain(bs)
            if s == 1:
                # device-side padded-row constant: one all-pad tile through
                # the exact same GEMM -> GELU -> square pipeline so c matches
                # padded-row outputs bitwise (emitted after group 0 so the
                # main GEMM stream starts as soon as xg0/w3 land; also
                # preloads the Sqrt ACT table during the main loop)
                pyc = ps_y.tile([128, 2, 512], f32, name="pyc", tag="py")
                nc.tensor.matmul(pyc[:, 0, 0:D], xpa_sb[:], w8_sb[:, 0, :, :],
                                 start=True, stop=False, perf_mode=PM.DoubleRow)
                nc.tensor.matmul(pyc[:, 0, 0:D], xpb_sb[:], w8_sb[0:KB, 1, :, :],
                                 start=False, stop=True, perf_mode=PM.DoubleRow)
                nc.scalar.activation(cyy[:, 0:D], pyc[:, 0, 0:D], AF.Gelu,
                                     scale=SCI)
                nc.vector.tensor_mul(cyy[:, D:2 * D], cyy[:, 0:D], cyy[:, 0:D])
                nc.scalar.copy(cc32[0:1, :], cyy[0:1, :])
                nc.gpsimd.dma_start(ccv_dram.ap()[:], cc32[0:1, :])
        qstat(NT - 2)
        qstat(NT - 1)
        sstat(NP - 1)
        drain(B_LOC - 1)

    nc.compile()
    return nc


def _prep_inputs(x, W, b):
    """Host prep: k-transpose x into grouped tiles, pack W^T k-tiles + bias row,
    precompute the padded-row GELU constant c."""
    import ml_dtypes
    fp8 = ml_dtypes.float8_e4m3fn
    bft = ml_dtypes.bfloat16
    x = np.ascontiguousarray(x, np.float32)
    W = np.asarray(W, np.float32)
    b = np.asarray(b, np.float32)

    x8 = x.astype(fp8)
    xr8 = x8.reshape(B, GPB, G, 128, D)  # [b,grp,g,tok,k]
    # m0: k = s*128 + kp  (k 0..255)
    xta = np.ascontiguousarray(
        xr8[..., 0:256].reshape(B, GPB, G, 128, 2, 128)
        .transpose(0, 1, 5, 2, 4, 3))    # [b,grp,kp,g,s,tok]
    # m1: k = 256 + s*KB + kp (kp<KB); k==300/301 -> 0 (bias handled in bf16)
    xtb = np.zeros((B, GPB, KB, G, 2, 128), fp8)
    xtb[:, :, :, :, 0, :] = xr8[..., 256:256 + KB].transpose(0, 1, 4, 2, 3)
    xtb[:, :, 0:D - 256 - KB, :, 1, :] = (
        xr8[..., 256 + KB:D].transpose(0, 1, 4, 2, 3))
    xtb[:, :, KB - 2:KB, :, 1, :] = fp8(1.0)   # bias ones rows
    shards_a = [
        np.ascontiguousarray(
            xta[c * B_LOC:(c + 1) * B_LOC].reshape(NG, 128, G * 2 * 128))
        for c in range(NCORES)
    ]
    shards_b = [
        np.ascontiguousarray(
            xtb[c * B_LOC:(c + 1) * B_LOC].reshape(NG, KB, G * 2 * 128))
        for c in range(NCORES)
    ]
    npad = (x[:, :, 0] == -1.0).sum(axis=1).astype(np.float64)  # [B]

    w16 = (W.T * WS).astype(fp8)      # [k, o], scaled into fp8 range
    w8 = np.zeros((128, 2, 2, D), fp8)
    w8[:, 0, 0, :] = w16[0:128]
    w8[:, 0, 1, :] = w16[128:256]
    w8[0:KB, 1, 0, :] = w16[256:256 + KB]
    w8[0:D - 256 - KB, 1, 1, :] = w16[256 + KB:D]
    # bias rides the two spare m1 rows (k "300"/"301") with fp8
    # error-feedback: b ~ fp8(WS*b) + fp8(WS*b - fp8(WS*b)), ~0.13% error
    b0 = (b * WS).astype(fp8)
    w8[KB - 2, 1, 1, :] = b0
    w8[KB - 1, 1, 1, :] = (b * WS - b0.astype(np.float64)).astype(fp8)

    # the all-padded-row tile: k<300 -> -1; ones on the two bias rows
    xpa = np.full((128, 2, 128), fp8(-1.0), fp8).reshape(128, 256)
    xpb = np.zeros((KB, 2, 128), fp8)
    xpb[:, 0, :] = fp8(-1.0)
    xpb[0:D - 256 - KB, 1, :] = fp8(-1.0)
    xpb[KB - 2:KB, 1, :] = fp8(1.0)
    xpb = xpb.reshape(KB, 256)
    return (shards_a, shards_b, w8.reshape(128, 2 * 2 * D), xpa, xpb, npad)


def kernel(x, W, b):
    from concourse.bass_utils import run_bass_kernel_spmd

    if "nc" not in _cache:
        _cache["nc"] = _build_nc()
    nc = _cache["nc"]

    import ml_dtypes
    sa, sb, w8, xpa, xpb, npad = _prep_inputs(x, W, b)
    on = np.ones((128, 1), ml_dtypes.bfloat16)
    in_maps = [{"xta": sa[c], "xtb": sb[c], "w8": w8,
                "xpa": xpa, "xpb": xpb, "on": on}
               for c in range(NCORES)]
    res = run_bass_kernel_spmd(nc, in_maps, core_ids=list(range(NCORES)))
    sums = np.concatenate(
        [res.results[c]["out"] for c in range(NCORES)], axis=0
    ).astype(np.float64)                      # [B, 600] = S | Q (unmasked)
    cc = np.concatenate(
        [np.repeat(res.results[c]["ccv"].astype(np.float64), B_LOC, axis=0)
         for c in range(NCORES)], axis=0)     # [B, 600] = c | c^2 per core
    n = (T - npad)[:, None]
    sv = sums - npad[:, None] * cc            # valid-token S | Q
    mean = sv[:, 0:D] / n
    var = (sv[:, D:2 * D] - n * mean * mean) / np.maximum(n - 1.0, 1.0)
    std = np.sqrt(np.maximum(var, 0.0))
    return np.concatenate([std, mean], axis=1).astype(np.float32)
